# revision 12
# baseline (speedup 1.0000x reference)
"""AlignmentModule kernel for 8 TRN2 NeuronCores (self-contained).

Reference computation (per batch):
  hc = conv1d_k1(relu(conv1d_k3(h^T)))^T            # (S, D) text features
  mc = conv1d_k1(relu(conv1d_k3(relu(conv1d_k3(m^T)))))^T  # (T, D) frame feats
  dist = sqrt(max(|hc|^2 + |mc|^2 - 2 hc.mc^T, 0))  # (S, T)
  dist = where(mask[:, None_s? per-s], dist, 0)
  out = log_softmax(-dist, axis=S)

Sharding: data-parallel over batch, 4 batches per core. All compute is done
in (channel, length) = channel-major layout so TensorE contracts over
channels; scores are computed as (t, s) tiles so the softmax over S runs
along the free axis. Output is produced as (B, T, S) and transposed on host.

The mask is folded in additively: a per-s vector g = -1e6 on masked-out
positions is accumulated into the squared distance via a K=1 matmul; the
clamp max(.,0) then zeroes dist exactly at masked positions, reproducing
`where(mask, dist, 0)`.
"""
import sys

import numpy as np

sys.path.insert(0, "/opt/trn_rl_repo")

import concourse.bass as bass  # noqa: E402
import concourse.tile as tile  # noqa: E402
from concourse import bacc, mybir  # noqa: E402
from concourse.bass import _add_dep_helper  # noqa: E402
from concourse.bass_utils import run_bass_kernel_spmd  # noqa: E402

F32 = mybir.dt.float32
F32R = mybir.dt.float32r
BF16 = mybir.dt.bfloat16
AF = mybir.ActivationFunctionType
ALU = mybir.AluOpType

B, S, T, D, F = 32, 512, 2048, 256, 80
NCORES = 8
BPC = B // NCORES  # batches per core
POS_BIG = 1.0e6
HBAR = 13.0

_NC_CACHE = {}


def _build_nc():
    nc = bacc.Bacc("TRN2", target_bir_lowering=False, debug=False,
                   num_devices=NCORES)

    # ---- DRAM parameters (per-core shapes) ----
    hTp_d = nc.dram_tensor("hTp", [BPC, D, S + 2], BF16, kind="ExternalInput").ap()
    m3_d = nc.dram_tensor("m3", [BPC, 3 * F, T + 4], BF16, kind="ExternalInput").ap()
    g_d = nc.dram_tensor("gvec", [BPC, S], F32, kind="ExternalInput").ap()
    w1_d = nc.dram_tensor("w1t", [128, 6 * D], BF16, kind="ExternalInput").ap()
    w2_d = nc.dram_tensor("w2t", [128, 2 * D], BF16, kind="ExternalInput").ap()
    f1_d = nc.dram_tensor("f1t", [120, 2 * D], BF16, kind="ExternalInput").ap()
    f2_d = nc.dram_tensor("f2t", [128, 6 * D], BF16, kind="ExternalInput").ap()
    f3_d = nc.dram_tensor("f3t", [128, 2 * D], BF16, kind="ExternalInput").ap()
    bias_d = nc.dram_tensor("biasp", [128, 10], F32, kind="ExternalInput").ap()
    q_d = nc.dram_tensor("qvec", [128, 2], BF16, kind="ExternalInput").ap()
    onesa_d = nc.dram_tensor("onesA", [128, 1], BF16, kind="ExternalInput").ap()
    onesb_d = nc.dram_tensor("onesB", [1, 128], BF16, kind="ExternalInput").ap()
    cb_d = nc.dram_tensor("cbt", [BPC, 128], F32, kind="ExternalInput").ap()
    out_d = nc.dram_tensor("out", [BPC, T, S], F32, kind="ExternalOutput").ap()
    lse_d = nc.dram_tensor("lse", [BPC, 128, 16], F32, kind="ExternalOutput").ap()

    with tile.TileContext(nc) as tc:
        _emit(nc, tc, hTp_d, m3_d, g_d, w1_d, w2_d, f1_d, f2_d, f3_d,
              bias_d, q_d, onesa_d, onesb_d, cb_d, out_d, lse_d)
    nc.compile()
    return nc


def _emit(nc, tc, hTp_d, m3_d, g_d, w1_d, w2_d, f1_d, f2_d, f3_d,
          bias_d, q_d, onesa_d, onesb_d, cb_d, out_d, lse_d):
    from contextlib import ExitStack
    ctx = ExitStack()
    with ctx:
        wp = ctx.enter_context(tc.tile_pool(name="weights", bufs=1))
        xp = ctx.enter_context(tc.tile_pool(name="acts", bufs=2))
        mcp = ctx.enter_context(tc.tile_pool(name="mc", bufs=8))
        scp = ctx.enter_context(tc.tile_pool(name="score", bufs=2))
        ps = ctx.enter_context(tc.tile_pool(name="ps", bufs=1, space="PSUM"))

        # ---- load weights/constants once ----
        w1 = wp.tile([128, 6 * D], BF16, tag="w1")
        w2 = wp.tile([128, 2 * D], BF16, tag="w2")
        f1 = wp.tile([120, 2 * D], BF16, tag="f1")
        f2 = wp.tile([128, 6 * D], BF16, tag="f2")
        f3 = wp.tile([128, 2 * D], BF16, tag="f3")
        bias = wp.tile([128, 10], F32, tag="bias")
        qv = wp.tile([128, 2], BF16, tag="qv")
        onesa = wp.tile([128, 1], BF16, tag="onesa")
        onesb = wp.tile([1, 128], BF16, tag="onesb")
        for t_, d_ in ((w1, w1_d), (w2, w2_d), (f1, f1_d), (f2, f2_d),
                       (f3, f3_d), (bias, bias_d), (qv, q_d),
                       (onesa, onesa_d), (onesb, onesb_d)):
            nc.sync.dma_start(t_[:], d_[:])

        def text_stack(b):
            X = xp.tile([128, 2, S + 2], BF16, tag="X")
            nc.sync.dma_start(
                X[:], hTp_d[b].rearrange("(c p) s -> p c s", p=128))
            gt = xp.tile([1, S], F32, tag="gt")
            nc.sync.dma_start(gt[:], g_d[b][None, :])

            X1 = xp.tile([128, 2, S], BF16, tag="X1")
            for o in range(2):
                p = ps.tile([128, S], F32, tag="p512", bufs=2)
                first = True
                for c in range(2):
                    for k in range(3):
                        nc.tensor.matmul(
                            p[:], w1[:, bass.ts(k * 2 + c, D)][:, bass.ts(o, 128)],
                            X[:, c, k:k + S], start=first, stop=(c == 1 and k == 2))
                        first = False
                nc.vector.tensor_scalar(X1[:, o, :], p[:],
                                        bias[:, 0 + o:1 + o], 0.0,
                                        ALU.add, ALU.max)

            hcT = xp.tile([128, 2, S], BF16, tag="hcT")
            for o in range(2):
                p = ps.tile([128, S], F32, tag="p512", bufs=2)
                for c in range(2):
                    nc.tensor.matmul(
                        p[:], w2[:, bass.ts(c, D)][:, bass.ts(o, 128)],
                        X1[:, c, :], start=(c == 0), stop=(c == 1))
                nc.vector.tensor_scalar(hcT[:, o, :], p[:],
                                        bias[:, 2 + o:3 + o], None, ALU.add)

            # |hc|^2 per s  (+ mask vector) -> hgr (1, S)
            hsq = xp.tile([128, 2, S], BF16, tag="hsq")
            nc.gpsimd.tensor_mul(hsq[:, :, :], hcT[:, :, :], hcT[:, :, :])
            hhp = ps.tile([1, S], F32, tag="row")
            for o in range(2):
                nc.tensor.matmul(hhp[:], onesa[:], hsq[:, o, :],
                                 start=(o == 0), stop=(o == 1))
            hgr = xp.tile([1, S], BF16, tag="hgr")
            nc.vector.tensor_add(hgr[:], hhp[:], gt[:])
            return hcT, hgr

        def feat_tiles(st, tts):
            b, mmP, M2C = st["b"], st["mmP"], st["M2C"]
            for tt in tts:
                M3 = xp.tile([120, 2, 514], BF16, tag="M3")
                nc.sync.dma_start(
                    M3[:],
                    m3_d[b].rearrange("(c p) t -> p c t", p=120)[
                        :, :, tt * 512: tt * 512 + 514])

                X2 = xp.tile([128, 2, 514], BF16, tag="X2")
                for o in range(2):
                    pa = ps.tile([128, 514], F32, tag="p514", bufs=1)
                    for c in range(2):
                        nc.tensor.matmul(
                            pa[:, 0:512],
                            f1[0:120, bass.ts(c, D)][:, bass.ts(o, 128)],
                            M3[:, c, 0:512], start=(c == 0), stop=(c == 1))
                    for c in range(2):
                        nc.tensor.matmul(
                            pa[:, 512:514],
                            f1[0:120, bass.ts(c, D)][:, bass.ts(o, 128)],
                            M3[:, c, 512:514], start=(c == 0), stop=(c == 1))
                    nc.vector.tensor_scalar(X2[:, o, :], pa[:],
                                            bias[:, 4 + o:5 + o], 0.0,
                                            ALU.add, ALU.max)
                if tt == 0:
                    nc.vector.memset(X2[:, :, 0:1], 0.0)
                if tt == 3:
                    nc.vector.memset(X2[:, :, 513:514], 0.0)

                X3 = xp.tile([128, 2, 512], BF16, tag="X3")
                for o in range(2):
                    p = ps.tile([128, 512], F32, tag="p512", bufs=2)
                    first = True
                    for c in range(2):
                        for k in range(3):
                            nc.tensor.matmul(
                                p[:], f2[:, bass.ts(k * 2 + c, D)][:, bass.ts(o, 128)],
                                X2[:, c, k:k + 512], start=first,
                                stop=(c == 1 and k == 2))
                            first = False
                    nc.vector.tensor_scalar(X3[:, o, :], p[:],
                                            bias[:, 6 + o:7 + o], 0.0,
                                            ALU.add, ALU.max)

                m2c = mcp.tile([128, 2, 512], BF16, tag="m2c")
                for o in range(2):
                    p = ps.tile([128, 512], F32, tag="p512", bufs=2)
                    for c in range(2):
                        nc.tensor.matmul(
                            p[:], f3[:, bass.ts(c, D)][:, bass.ts(o, 128)],
                            X3[:, c, :], start=(c == 0), stop=(c == 1))
                    nc.vector.tensor_scalar(m2c[:, o, :], p[:],
                                            bias[:, 8 + o:9 + o], None,
                                            ALU.add)
                M2C.append(m2c)

                # |mc|^2 for the 4 t-subtiles of this T-tile
                msq = xp.tile([128, 2, 512], BF16, tag="msq")
                nc.gpsimd.tensor_mul(msq[:, :, :], m2c[:, :, :], m2c[:, :, :])
                for j in range(4):
                    col = tt * 4 + j
                    for o in range(2):
                        nc.tensor.matmul(
                            mmP[:, 2 * col:2 * col + 2],
                            msq[:, o, bass.ts(j, 128)], qv[:],
                            start=(o == 0), stop=(o == 1))
                nc.vector.tensor_scalar_add(
                    st["mmv"][:, 8 * tt:8 * tt + 8],
                    mmP[:, 8 * tt:8 * tt + 8], HBAR)

        # --- score stages for batch b (state dict carries tiles) ---
        def score_mm_sqrt(st, rng):
            hcT, hgr, mmv, M2C = st["hcT"], st["hgr"], st["mmv"], st["M2C"]
            for j2 in rng:
                tt, j = j2 // 4, j2 % 4
                m2c = M2C[tt]
                p = ps.tile([128, 512], F32, tag="pscore", bufs=2)
                for o in range(2):
                    nc.tensor.matmul(p[:], m2c[:, o, bass.ts(j, 128)],
                                     hcT[:, o, :], start=(o == 0), stop=False)
                nc.tensor.matmul(p[:], onesb[:], hgr[:], start=False, stop=True)
                dist = scp.tile([128, 512], F32, tag="dist", bufs=16)
                sq_i = nc.scalar.activation(dist[:], p[:], AF.Sqrt,
                                            bias=mmv[:, 2 * j2:2 * j2 + 1],
                                            scale=1.0)
                for t_ in st.get("act_barrier", ()):
                    _add_dep_helper(sq_i.ins, t_.ins, sync=True,
                                    reason="ACT wave ordering")
                st["dists"][j2] = dist

        def score_exp(st, rng):
            if "ssum16" not in st:
                ssum16 = scp.tile([128, 16], F32, tag="ssum", bufs=2)
                st["ssum16"] = ssum16
            for j2 in rng:
                e = scp.tile([128, 512], F32, tag="e", bufs=2)
                e_i = nc.scalar.activation(e[:], st["dists"][j2][:], AF.Exp,
                                           scale=-1.0,
                                           accum_out=st["ssum16"][:, j2:j2 + 1])
                st.setdefault("act_insts", []).append(e_i)

        def score_ln_final(st, rng):
            b = st["b"]
            if "lse16" not in st:
                lse16 = scp.tile([128, 16], F32, tag="lse", bufs=2)
                st["lse16"] = lse16
            lse16 = st["lse16"]
            lo, hi = rng.start, rng.stop
            ln_i = nc.scalar.activation(lse16[:, lo:hi], st["ssum16"][:, lo:hi],
                                        AF.Ln, bias=st["cbt"][:], scale=1.0)
            st.setdefault("act_insts", []).append(ln_i)
            for j2 in rng:
                obuf = scp.tile([128, 512], F32, tag="obuf", bufs=4)
                nc.gpsimd.tensor_scalar(obuf[:], st["dists"][j2][:],
                                        lse16[:, j2:j2 + 1],
                                        -1.0, ALU.add, ALU.mult)
                nc.sync.dma_start(out_d[b, bass.ts(j2, 128), :], obuf[:])
            if hi == 16:
                nc.sync.dma_start(lse_d[b], lse16[:])

        def conv_phase(b):
            hcT, hgr = text_stack(b)
            mmP = ps.tile([128, 32], F32, tag="mm")
            M2C = []
            cbt = xp.tile([128, 1], F32, tag="cbt")
            nc.sync.dma_start(cbt[:], cb_d[b][:, None])
            mmv = xp.tile([128, 32], F32, tag="mmv")
            st = {"b": b, "hcT": hcT, "hgr": hgr, "M2C": M2C, "mmP": mmP,
                  "cbt": cbt, "mmv": mmv, "dists": {}}
            return st

        # --- software pipeline over batches ---
        prev = None
        for b in range(BPC):
            st = conv_phase(b)
            feat_tiles(st, [0])
            if prev is not None:
                score_exp(prev, range(0, 16))
            feat_tiles(st, [1])
            if prev is not None:
                score_ln_final(prev, range(0, 8))
                score_ln_final(prev, range(8, 16))
            if prev is not None:
                st["act_barrier"] = prev.get("act_insts", [])
            feat_tiles(st, [2])
            score_mm_sqrt(st, range(0, 4))
            score_mm_sqrt(st, range(4, 8))
            feat_tiles(st, [3])
            score_mm_sqrt(st, range(8, 12))
            score_mm_sqrt(st, range(12, 16))
            prev = st
        score_exp(prev, range(0, 8))
        score_ln_final(prev, range(0, 8))
        score_exp(prev, range(8, 16))
        score_ln_final(prev, range(8, 16))


def _prep(h, m, mask, tw1, tb1, tw2, tb2, fw1, fb1, fw2, fb2, fw3, fb3):
    f32 = np.float32
    h = np.asarray(h, f32)
    m = np.asarray(m, f32)
    mask = np.asarray(mask)

    hT = h.transpose(0, 2, 1)                      # (B, D, S)
    hTp = np.zeros((B, D, S + 2), f32)
    hTp[:, :, 1:S + 1] = hT

    mT = m.transpose(0, 2, 1)                      # (B, F, T)
    mTpad = np.zeros((B, F, T + 4), f32)
    mTpad[:, :, 2:T + 2] = mT
    m3 = np.zeros((B, 3 * F, T + 4), f32)
    for k in range(3):
        m3[:, k * F:(k + 1) * F, :T + 4 - k] = mTpad[:, :, k:]

    gvec = np.where(mask, -HBAR, POS_BIG).astype(f32)
    lengths = mask.sum(1)
    cbt = np.repeat((S - lengths).astype(f32)[:, None], 128, axis=1)

    tw1 = np.asarray(tw1, f32); tw2 = np.asarray(tw2, f32)
    fw1 = np.asarray(fw1, f32); fw2 = np.asarray(fw2, f32)
    fw3 = np.asarray(fw3, f32)

    # lhsT layouts: [K(cin) partition, (tap, chunk, cout) free]
    w1t = np.ascontiguousarray(
        tw1.transpose(2, 1, 0).reshape(3, 2, 128, D)
        .transpose(2, 0, 1, 3).reshape(128, 6 * D))
    w2t = np.ascontiguousarray(
        tw2[:, :, 0].T.reshape(2, 128, D).transpose(1, 0, 2).reshape(128, 2 * D))
    W1s = fw1.transpose(2, 1, 0).reshape(3 * F, D)      # (240, 256)
    f1t = np.ascontiguousarray(
        W1s.reshape(2, 120, D).transpose(1, 0, 2).reshape(120, 2 * D))
    f2t = np.ascontiguousarray(
        fw2.transpose(2, 1, 0).reshape(3, 2, 128, D)
        .transpose(2, 0, 1, 3).reshape(128, 6 * D))
    f3t = np.ascontiguousarray(
        (-2.0 * fw3[:, :, 0]).T.reshape(2, 128, D)
        .transpose(1, 0, 2).reshape(128, 2 * D))

    biasp = np.zeros((128, 10), f32)
    for i, bv in enumerate((tb1, tb2, fb1, fb2, -2.0 * np.asarray(fb3, f32))):
        bv = np.asarray(bv, f32).reshape(2, 128).T
        biasp[:, 2 * i:2 * i + 2] = bv
    qvec = np.zeros((128, 2), f32); qvec[:, 0] = 0.25
    onesA = np.ones((128, 1), f32)
    onesB = np.ones((1, 128), f32)

    import ml_dtypes
    bf16 = ml_dtypes.bfloat16
    shared = dict(w1t=w1t.astype(bf16), w2t=w2t.astype(bf16),
                  f1t=f1t.astype(bf16), f2t=f2t.astype(bf16),
                  f3t=f3t.astype(bf16), biasp=biasp,
                  qvec=qvec.astype(bf16), onesA=onesA.astype(bf16),
                  onesB=onesB.astype(bf16))
    hTp16 = hTp.astype(bf16)
    m316 = m3.astype(bf16)
    in_maps = []
    for i in range(NCORES):
        sl = slice(i * BPC, (i + 1) * BPC)
        in_maps.append(dict(
            hTp=np.ascontiguousarray(hTp16[sl]),
            m3=np.ascontiguousarray(m316[sl]),
            gvec=np.ascontiguousarray(gvec[sl]),
            cbt=np.ascontiguousarray(cbt[sl]),
            **shared))
    return in_maps


def get_nc():
    if "nc" not in _NC_CACHE:
        _NC_CACHE["nc"] = _build_nc()
    return _NC_CACHE["nc"]


def run(in_maps, **kw):
    nc = get_nc()
    return run_bass_kernel_spmd(nc, in_maps, core_ids=list(range(NCORES)), **kw)


def assemble(res, mask):
    full = np.concatenate([res.results[i]["out"] for i in range(NCORES)], axis=0)
    out = np.ascontiguousarray(full.transpose(0, 2, 1))  # (B, S, T)
    lse = np.concatenate([res.results[i]["lse"] for i in range(NCORES)], axis=0)
    # lse[b, p, j2] is lse at t = j2*128 + p -> (B, T)
    lse_t = lse.transpose(0, 2, 1).reshape(B, T)
    mask = np.asarray(mask)
    for b in range(B):
        out[b, ~mask[b], :] = -lse_t[b][None, :]
    return out


def kernel(**inputs):
    in_maps = _prep(**inputs)
    res = run(in_maps)
    return assemble(res, inputs["mask"])


# revision 13
# speedup vs baseline: 1.2149x; 1.2149x over previous
"""AlignmentModule kernel for 8 TRN2 NeuronCores (self-contained).

Reference computation (per batch):
  hc = conv1d_k1(relu(conv1d_k3(h^T)))^T            # (S, D) text features
  mc = conv1d_k1(relu(conv1d_k3(relu(conv1d_k3(m^T)))))^T  # (T, D) frame feats
  dist = sqrt(max(|hc|^2 + |mc|^2 - 2 hc.mc^T, 0))  # (S, T)
  dist = where(mask[:, None_s? per-s], dist, 0)
  out = log_softmax(-dist, axis=S)

Sharding: data-parallel over batch, 4 batches per core. All compute is done
in (channel, length) = channel-major layout so TensorE contracts over
channels; scores are computed as (t, s) tiles so the softmax over S runs
along the free axis. Output is produced as (B, T, S) and transposed on host.

The mask is folded in additively: a per-s vector g = -1e6 on masked-out
positions is accumulated into the squared distance via a K=1 matmul; the
clamp max(.,0) then zeroes dist exactly at masked positions, reproducing
`where(mask, dist, 0)`.
"""
import sys

import numpy as np

sys.path.insert(0, "/opt/trn_rl_repo")

import concourse.bass as bass  # noqa: E402
import concourse.tile as tile  # noqa: E402
from concourse import bacc, mybir  # noqa: E402
from concourse.bass import _add_dep_helper  # noqa: E402
from concourse.bass_utils import run_bass_kernel_spmd  # noqa: E402

F32 = mybir.dt.float32
F32R = mybir.dt.float32r
BF16 = mybir.dt.bfloat16
AF = mybir.ActivationFunctionType
ALU = mybir.AluOpType

B, S, T, D, F = 32, 512, 2048, 256, 80
NCORES = 8
BPC = B // NCORES  # batches per core
POS_BIG = 1.0e6
HBAR = 13.0

_NC_CACHE = {}


def _build_nc():
    nc = bacc.Bacc("TRN2", target_bir_lowering=False, debug=False,
                   num_devices=NCORES)

    # ---- DRAM parameters (per-core shapes) ----
    hTp_d = nc.dram_tensor("hTp", [BPC, D, S + 2], BF16, kind="ExternalInput").ap()
    m3_d = nc.dram_tensor("m3", [BPC, 3 * F, T + 4], BF16, kind="ExternalInput").ap()
    g_d = nc.dram_tensor("gvec", [BPC, S], F32, kind="ExternalInput").ap()
    w1_d = nc.dram_tensor("w1t", [128, 6 * D], BF16, kind="ExternalInput").ap()
    w2_d = nc.dram_tensor("w2t", [128, 2 * D], BF16, kind="ExternalInput").ap()
    f1_d = nc.dram_tensor("f1t", [120, 2 * D], BF16, kind="ExternalInput").ap()
    f2_d = nc.dram_tensor("f2t", [128, 6 * D], BF16, kind="ExternalInput").ap()
    f3_d = nc.dram_tensor("f3t", [128, 2 * D], BF16, kind="ExternalInput").ap()
    bias_d = nc.dram_tensor("biasp", [128, 10], F32, kind="ExternalInput").ap()
    q_d = nc.dram_tensor("qvec", [128, 2], BF16, kind="ExternalInput").ap()
    onesa_d = nc.dram_tensor("onesA", [128, 1], BF16, kind="ExternalInput").ap()
    onesb_d = nc.dram_tensor("onesB", [1, 128], BF16, kind="ExternalInput").ap()
    cb_d = nc.dram_tensor("cbt", [BPC, 128], F32, kind="ExternalInput").ap()
    out_d = nc.dram_tensor("out", [BPC, T, S], F32, kind="ExternalOutput").ap()
    lse_d = nc.dram_tensor("lse", [BPC, 128, 16], F32, kind="ExternalOutput").ap()

    with tile.TileContext(nc) as tc:
        _emit(nc, tc, hTp_d, m3_d, g_d, w1_d, w2_d, f1_d, f2_d, f3_d,
              bias_d, q_d, onesa_d, onesb_d, cb_d, out_d, lse_d)
    nc.compile()
    return nc


def _emit(nc, tc, hTp_d, m3_d, g_d, w1_d, w2_d, f1_d, f2_d, f3_d,
          bias_d, q_d, onesa_d, onesb_d, cb_d, out_d, lse_d):
    from contextlib import ExitStack
    ctx = ExitStack()
    with ctx:
        wp = ctx.enter_context(tc.tile_pool(name="weights", bufs=1))
        xp = ctx.enter_context(tc.tile_pool(name="acts", bufs=2))
        mcp = ctx.enter_context(tc.tile_pool(name="mc", bufs=8))
        scp = ctx.enter_context(tc.tile_pool(name="score", bufs=2))
        ps = ctx.enter_context(tc.tile_pool(name="ps", bufs=1, space="PSUM"))

        # ---- load weights/constants once ----
        w1 = wp.tile([128, 6 * D], BF16, tag="w1")
        w2 = wp.tile([128, 2 * D], BF16, tag="w2")
        f1 = wp.tile([120, 2 * D], BF16, tag="f1")
        f2 = wp.tile([128, 6 * D], BF16, tag="f2")
        f3 = wp.tile([128, 2 * D], BF16, tag="f3")
        bias = wp.tile([128, 10], F32, tag="bias")
        qv = wp.tile([128, 2], BF16, tag="qv")
        onesa = wp.tile([128, 1], BF16, tag="onesa")
        onesb = wp.tile([1, 128], BF16, tag="onesb")
        for t_, d_ in ((w1, w1_d), (w2, w2_d), (f1, f1_d), (f2, f2_d),
                       (f3, f3_d), (bias, bias_d), (qv, q_d),
                       (onesa, onesa_d), (onesb, onesb_d)):
            nc.sync.dma_start(t_[:], d_[:])

        def text_stack(b):
            X = xp.tile([128, 2, S + 2], BF16, tag="X")
            nc.sync.dma_start(
                X[:], hTp_d[b].rearrange("(c p) s -> p c s", p=128))
            gt = xp.tile([1, S], F32, tag="gt")
            nc.sync.dma_start(gt[:], g_d[b][None, :])

            X1 = xp.tile([128, 2, S], BF16, tag="X1")
            for o in range(2):
                p = ps.tile([128, S], F32, tag="p512", bufs=2)
                first = True
                for c in range(2):
                    for k in range(3):
                        nc.tensor.matmul(
                            p[:], w1[:, bass.ts(k * 2 + c, D)][:, bass.ts(o, 128)],
                            X[:, c, k:k + S], start=first, stop=(c == 1 and k == 2))
                        first = False
                nc.vector.tensor_scalar(X1[:, o, :], p[:],
                                        bias[:, 0 + o:1 + o], 0.0,
                                        ALU.add, ALU.max)

            hcT = xp.tile([128, 2, S], BF16, tag="hcT")
            for o in range(2):
                p = ps.tile([128, S], F32, tag="p512", bufs=2)
                for c in range(2):
                    nc.tensor.matmul(
                        p[:], w2[:, bass.ts(c, D)][:, bass.ts(o, 128)],
                        X1[:, c, :], start=(c == 0), stop=(c == 1))
                nc.vector.tensor_scalar(hcT[:, o, :], p[:],
                                        bias[:, 2 + o:3 + o], None, ALU.add)

            # |hc|^2 per s  (+ mask vector) -> hgr (1, S)
            hsq = xp.tile([128, 2, S], BF16, tag="hsq")
            nc.gpsimd.tensor_mul(hsq[:, :, :], hcT[:, :, :], hcT[:, :, :])
            hhp = ps.tile([1, S], F32, tag="row")
            for o in range(2):
                nc.tensor.matmul(hhp[:], onesa[:], hsq[:, o, :],
                                 start=(o == 0), stop=(o == 1))
            hgr = xp.tile([1, S], BF16, tag="hgr")
            nc.vector.tensor_add(hgr[:], hhp[:], gt[:])
            return hcT, hgr

        def feat_tiles(st, tts):
            b, mmP, M2C = st["b"], st["mmP"], st["M2C"]
            for tt in tts:
                M3 = xp.tile([120, 2, 514], BF16, tag="M3")
                nc.sync.dma_start(
                    M3[:],
                    m3_d[b].rearrange("(c p) t -> p c t", p=120)[
                        :, :, tt * 512: tt * 512 + 514])

                X2 = xp.tile([128, 2, 514], BF16, tag="X2")
                for o in range(2):
                    pa = ps.tile([128, 514], F32, tag="p514", bufs=1)
                    for c in range(2):
                        nc.tensor.matmul(
                            pa[:, 0:512],
                            f1[0:120, bass.ts(c, D)][:, bass.ts(o, 128)],
                            M3[:, c, 0:512], start=(c == 0), stop=(c == 1))
                    for c in range(2):
                        nc.tensor.matmul(
                            pa[:, 512:514],
                            f1[0:120, bass.ts(c, D)][:, bass.ts(o, 128)],
                            M3[:, c, 512:514], start=(c == 0), stop=(c == 1))
                    nc.vector.tensor_scalar(X2[:, o, :], pa[:],
                                            bias[:, 4 + o:5 + o], 0.0,
                                            ALU.add, ALU.max)
                if tt == 0:
                    nc.vector.memset(X2[:, :, 0:1], 0.0)
                if tt == 3:
                    nc.vector.memset(X2[:, :, 513:514], 0.0)

                X3 = xp.tile([128, 2, 512], BF16, tag="X3")
                for o in range(2):
                    p = ps.tile([128, 512], F32, tag="p512", bufs=2)
                    first = True
                    for c in range(2):
                        for k in range(3):
                            nc.tensor.matmul(
                                p[:], f2[:, bass.ts(k * 2 + c, D)][:, bass.ts(o, 128)],
                                X2[:, c, k:k + 512], start=first,
                                stop=(c == 1 and k == 2))
                            first = False
                    nc.vector.tensor_scalar(X3[:, o, :], p[:],
                                            bias[:, 6 + o:7 + o], 0.0,
                                            ALU.add, ALU.max)

                m2c = mcp.tile([128, 2, 512], BF16, tag="m2c")
                for o in range(2):
                    p = ps.tile([128, 512], F32, tag="p512", bufs=2)
                    for c in range(2):
                        nc.tensor.matmul(
                            p[:], f3[:, bass.ts(c, D)][:, bass.ts(o, 128)],
                            X3[:, c, :], start=(c == 0), stop=(c == 1))
                    nc.vector.tensor_scalar(m2c[:, o, :], p[:],
                                            bias[:, 8 + o:9 + o], None,
                                            ALU.add)
                M2C.append(m2c)

                # |mc|^2 for the 4 t-subtiles of this T-tile
                msq = xp.tile([128, 2, 512], BF16, tag="msq")
                nc.gpsimd.tensor_mul(msq[:, :, :], m2c[:, :, :], m2c[:, :, :])
                for j in range(4):
                    col = tt * 4 + j
                    for o in range(2):
                        nc.tensor.matmul(
                            mmP[:, 2 * col:2 * col + 2],
                            msq[:, o, bass.ts(j, 128)], qv[:],
                            start=(o == 0), stop=(o == 1))
                nc.vector.tensor_scalar_add(
                    st["mmv"][:, 8 * tt:8 * tt + 8],
                    mmP[:, 8 * tt:8 * tt + 8], HBAR)

        # --- score stages for batch b (state dict carries tiles) ---
        def score_mm_sqrt(st, rng):
            hcT, hgr, mmv, M2C = st["hcT"], st["hgr"], st["mmv"], st["M2C"]
            for j2 in rng:
                tt, j = j2 // 4, j2 % 4
                m2c = M2C[tt]
                p = ps.tile([128, 512], F32, tag="pscore", bufs=2)
                for o in range(2):
                    nc.tensor.matmul(p[:], m2c[:, o, bass.ts(j, 128)],
                                     hcT[:, o, :], start=(o == 0), stop=False)
                nc.tensor.matmul(p[:], onesb[:], hgr[:], start=False, stop=True)
                dist = scp.tile([128, 512], F32, tag="dist", bufs=16)
                sq_i = nc.scalar.activation(dist[:], p[:], AF.Sqrt,
                                            bias=mmv[:, 2 * j2:2 * j2 + 1],
                                            scale=1.0)
                for t_ in st.get("act_barrier", ()):
                    _add_dep_helper(sq_i.ins, t_.ins, sync=True,
                                    reason="ACT wave ordering")
                st.setdefault("sqrt_insts", []).append(sq_i)
                st["dists"][j2] = dist

        def score_exp(st, rng):
            if "ssum16" not in st:
                ssum16 = scp.tile([128, 16], F32, tag="ssum", bufs=2)
                st["ssum16"] = ssum16
            for j2 in rng:
                e = scp.tile([128, 512], F32, tag="e", bufs=2)
                e_i = nc.scalar.activation(e[:], st["dists"][j2][:], AF.Exp,
                                           scale=-1.0,
                                           accum_out=st["ssum16"][:, j2:j2 + 1])
                _add_dep_helper(e_i.ins, st["sqrt_insts"][-1].ins, sync=True,
                                reason="exp wave after sqrt wave")
                st.setdefault("act_insts", []).append(e_i)

        def score_ln_final(st, rng):
            b = st["b"]
            if "lse16" not in st:
                lse16 = scp.tile([128, 16], F32, tag="lse", bufs=2)
                st["lse16"] = lse16
            lse16 = st["lse16"]
            lo, hi = rng.start, rng.stop
            ln_i = nc.scalar.activation(lse16[:, lo:hi], st["ssum16"][:, lo:hi],
                                        AF.Ln, bias=st["cbt"][:], scale=1.0)
            st.setdefault("act_insts", []).append(ln_i)
            for j2 in rng:
                obuf = scp.tile([128, 512], F32, tag="obuf", bufs=4)
                nc.gpsimd.tensor_scalar(obuf[:], st["dists"][j2][:],
                                        lse16[:, j2:j2 + 1],
                                        -1.0, ALU.add, ALU.mult)
                nc.sync.dma_start(out_d[b, bass.ts(j2, 128), :], obuf[:])
            if hi == 16:
                nc.sync.dma_start(lse_d[b], lse16[:])

        def conv_phase(b):
            hcT, hgr = text_stack(b)
            mmP = ps.tile([128, 32], F32, tag="mm")
            M2C = []
            cbt = xp.tile([128, 1], F32, tag="cbt")
            nc.sync.dma_start(cbt[:], cb_d[b][:, None])
            mmv = xp.tile([128, 32], F32, tag="mmv")
            st = {"b": b, "hcT": hcT, "hgr": hgr, "M2C": M2C, "mmP": mmP,
                  "cbt": cbt, "mmv": mmv, "dists": {}}
            return st

        # --- software pipeline over batches ---
        prev = None
        for b in range(BPC):
            st = conv_phase(b)
            feat_tiles(st, [0])
            if prev is not None:
                score_exp(prev, range(0, 16))
            feat_tiles(st, [1])
            if prev is not None:
                score_ln_final(prev, range(0, 8))
                score_ln_final(prev, range(8, 16))
            if prev is not None:
                st["act_barrier"] = prev.get("act_insts", [])
            feat_tiles(st, [2])
            score_mm_sqrt(st, range(0, 4))
            score_mm_sqrt(st, range(4, 8))
            feat_tiles(st, [3])
            score_mm_sqrt(st, range(8, 12))
            score_mm_sqrt(st, range(12, 16))
            prev = st
        score_exp(prev, range(0, 8))
        score_ln_final(prev, range(0, 8))
        score_exp(prev, range(8, 16))
        score_ln_final(prev, range(8, 16))


def _prep(h, m, mask, tw1, tb1, tw2, tb2, fw1, fb1, fw2, fb2, fw3, fb3):
    f32 = np.float32
    h = np.asarray(h, f32)
    m = np.asarray(m, f32)
    mask = np.asarray(mask)

    hT = h.transpose(0, 2, 1)                      # (B, D, S)
    hTp = np.zeros((B, D, S + 2), f32)
    hTp[:, :, 1:S + 1] = hT

    mT = m.transpose(0, 2, 1)                      # (B, F, T)
    mTpad = np.zeros((B, F, T + 4), f32)
    mTpad[:, :, 2:T + 2] = mT
    m3 = np.zeros((B, 3 * F, T + 4), f32)
    for k in range(3):
        m3[:, k * F:(k + 1) * F, :T + 4 - k] = mTpad[:, :, k:]

    gvec = np.where(mask, -HBAR, POS_BIG).astype(f32)
    lengths = mask.sum(1)
    cbt = np.repeat((S - lengths).astype(f32)[:, None], 128, axis=1)

    tw1 = np.asarray(tw1, f32); tw2 = np.asarray(tw2, f32)
    fw1 = np.asarray(fw1, f32); fw2 = np.asarray(fw2, f32)
    fw3 = np.asarray(fw3, f32)

    # lhsT layouts: [K(cin) partition, (tap, chunk, cout) free]
    w1t = np.ascontiguousarray(
        tw1.transpose(2, 1, 0).reshape(3, 2, 128, D)
        .transpose(2, 0, 1, 3).reshape(128, 6 * D))
    w2t = np.ascontiguousarray(
        tw2[:, :, 0].T.reshape(2, 128, D).transpose(1, 0, 2).reshape(128, 2 * D))
    W1s = fw1.transpose(2, 1, 0).reshape(3 * F, D)      # (240, 256)
    f1t = np.ascontiguousarray(
        W1s.reshape(2, 120, D).transpose(1, 0, 2).reshape(120, 2 * D))
    f2t = np.ascontiguousarray(
        fw2.transpose(2, 1, 0).reshape(3, 2, 128, D)
        .transpose(2, 0, 1, 3).reshape(128, 6 * D))
    f3t = np.ascontiguousarray(
        (-2.0 * fw3[:, :, 0]).T.reshape(2, 128, D)
        .transpose(1, 0, 2).reshape(128, 2 * D))

    biasp = np.zeros((128, 10), f32)
    for i, bv in enumerate((tb1, tb2, fb1, fb2, -2.0 * np.asarray(fb3, f32))):
        bv = np.asarray(bv, f32).reshape(2, 128).T
        biasp[:, 2 * i:2 * i + 2] = bv
    qvec = np.zeros((128, 2), f32); qvec[:, 0] = 0.25
    onesA = np.ones((128, 1), f32)
    onesB = np.ones((1, 128), f32)

    import ml_dtypes
    bf16 = ml_dtypes.bfloat16
    shared = dict(w1t=w1t.astype(bf16), w2t=w2t.astype(bf16),
                  f1t=f1t.astype(bf16), f2t=f2t.astype(bf16),
                  f3t=f3t.astype(bf16), biasp=biasp,
                  qvec=qvec.astype(bf16), onesA=onesA.astype(bf16),
                  onesB=onesB.astype(bf16))
    hTp16 = hTp.astype(bf16)
    m316 = m3.astype(bf16)
    in_maps = []
    for i in range(NCORES):
        sl = slice(i * BPC, (i + 1) * BPC)
        in_maps.append(dict(
            hTp=np.ascontiguousarray(hTp16[sl]),
            m3=np.ascontiguousarray(m316[sl]),
            gvec=np.ascontiguousarray(gvec[sl]),
            cbt=np.ascontiguousarray(cbt[sl]),
            **shared))
    return in_maps


def get_nc():
    if "nc" not in _NC_CACHE:
        _NC_CACHE["nc"] = _build_nc()
    return _NC_CACHE["nc"]


def run(in_maps, **kw):
    nc = get_nc()
    return run_bass_kernel_spmd(nc, in_maps, core_ids=list(range(NCORES)), **kw)


def assemble(res, mask):
    full = np.concatenate([res.results[i]["out"] for i in range(NCORES)], axis=0)
    out = np.ascontiguousarray(full.transpose(0, 2, 1))  # (B, S, T)
    lse = np.concatenate([res.results[i]["lse"] for i in range(NCORES)], axis=0)
    # lse[b, p, j2] is lse at t = j2*128 + p -> (B, T)
    lse_t = lse.transpose(0, 2, 1).reshape(B, T)
    mask = np.asarray(mask)
    for b in range(B):
        out[b, ~mask[b], :] = -lse_t[b][None, :]
    return out


def kernel(**inputs):
    in_maps = _prep(**inputs)
    res = run(in_maps)
    return assemble(res, inputs["mask"])
